# revision 28
# baseline (speedup 1.0000x reference)
"""Trainium2 Bass kernel for nn_AdaptedTransformerBlock (dense transformer block,
cross-attention + FFN) running SPMD on 8 NeuronCores.

Sharding (data parallel, zero collectives): core c handles batch c//2 and
query rows (c%2)*512..+512. K/V for a batch's context are computed
redundantly on the pair of cores sharing that batch.

Precision: the attention path (Q/K/V/O projections, P@V) runs in fp8-e4m3
with DoubleRow double-pumped matmuls; QK^T runs in bf16 (it is output-bound,
fp8 would not speed it up); the FFN runs in bf16 (fp8 there costs too much
accuracy). exp() is emitted with a 1/32 output scale folded into the
activation bias so softmax weights fit fp8 range; the scale cancels in the
sum-exp normalization. All accumulation is f32 in PSUM.
"""

import numpy as np
import ml_dtypes


import concourse.bass as bass
import concourse.mybir as mybir
import concourse.tile as tile
from concourse import bacc
from concourse.masks import make_identity

F32 = mybir.dt.float32
BF16 = mybir.dt.bfloat16
FP8 = mybir.dt.float8e4
AF = mybir.ActivationFunctionType
DR = mybir.MatmulPerfMode.DoubleRow

P = 128
DIM = 1024
INNER = 1024
HEADS = 16
DH = 64
FF = 4096
LQ = 512          # queries per core
LK = 2048
KO = DIM // P     # 8 contraction subtiles over dim
IC = INNER // P   # 8 inner chunks
KC = LK // P      # 16 key-position chunks
QO = LQ // P      # 4 query chunks
FC = FF // P      # 32 ffn chunks
SCALE = 0.125     # 1/sqrt(64)
LN_INV32 = float(np.log(1.0 / 32.0))  # exp output scale, cancels in softmax


def build(nc: bass.Bass):
    # ---- kernel I/O -------------------------------------------------------
    # fp8 inputs are pre-shuffled on the host to partition-major layouts
    # so each partition's data is contiguous in DRAM (8KB descriptors
    # instead of 1KB — the initial loads are descriptor-rate-bound)
    xT8_ext = nc.dram_tensor("xT8", [P, KO, LQ], FP8, kind="ExternalInput")
    xT_ext = nc.dram_tensor("xT", [P, KO, LQ], BF16, kind="ExternalInput")
    ctxT_ext = nc.dram_tensor("ctxT8", [LK // 512, P, KO, 512], FP8, kind="ExternalInput")
    wq_ext = nc.dram_tensor("Wq", [P, KO, INNER], FP8, kind="ExternalInput")
    wk_ext = nc.dram_tensor("Wk", [P, KO, INNER], FP8, kind="ExternalInput")
    wv_ext = nc.dram_tensor("Wv", [P, KO, INNER], FP8, kind="ExternalInput")
    wo_ext = nc.dram_tensor("Wo", [P, KO, INNER], FP8, kind="ExternalInput")
    bo_ext = nc.dram_tensor("bo", [DIM], F32, kind="ExternalInput")
    w1_ext = nc.dram_tensor("W1", [DIM, FF], BF16, kind="ExternalInput")
    b1_ext = nc.dram_tensor("b1", [FF], F32, kind="ExternalInput")
    w2_ext = nc.dram_tensor("W2", [FF, DIM], BF16, kind="ExternalInput")
    b2_ext = nc.dram_tensor("b2", [DIM], F32, kind="ExternalInput")
    out_ext = nc.dram_tensor("outT", [P, KO, LQ], BF16, kind="ExternalOutput")

    xT8_t = xT8_ext[:]                                 # [128,8,512]
    xT_t = xT_ext[:]                                   # [128,8,512]
    ctxT_t = ctxT_ext[:]                               # [4,128,8,512]
    wq_t = wq_ext[:]                                   # [128,8,1024]
    wk_t = wk_ext[:]
    wv_t = wv_ext[:]
    wo_t = wo_ext[:]
    w1_t = w1_ext[:].rearrange("(ko p) f -> p ko f", p=P)      # [128,8,4096]
    w2_t = w2_ext[:].rearrange("(ks p) d -> p ks d", p=P)      # [128,32,1024]
    bo_t = bo_ext[:].rearrange("(c p) -> p c", p=P)            # [128,8]
    b1_t = b1_ext[:].rearrange("(c p) -> p c", p=P)            # [128,32]
    out_t = out_ext[:]                                 # [128,8,512] partition-major

    with tile.TileContext(nc) as tc:
        with (
            tc.tile_pool(name="const", bufs=1) as const,
            tc.tile_pool(name="dram", bufs=1, space="DRAM") as dram,
            tc.tile_pool(name="pA", bufs=1) as pA,
            tc.tile_pool(name="pKT", bufs=1) as pKT,
            tc.tile_pool(name="pV", bufs=1) as pV,
            tc.tile_pool(name="pQT", bufs=1) as pQT,
            tc.tile_pool(name="pXT", bufs=1) as pXT,
            tc.tile_pool(name="pX8", bufs=1) as pX8,
            tc.tile_pool(name="pPT", bufs=12) as pPT,
            tc.tile_pool(name="pOT", bufs=1) as pOT,
            tc.tile_pool(name="pO8", bufs=1) as pO8,
            tc.tile_pool(name="pHT", bufs=1) as pHT,
            tc.tile_pool(name="pX1", bufs=1) as pX1,
            tc.tile_pool(name="wres", bufs=3) as wres,
            tc.tile_pool(name="wstr", bufs=11) as wstr,
            tc.tile_pool(name="io32", bufs=1) as io32,
            tc.tile_pool(name="rp", bufs=2) as rp,
            tc.tile_pool(name="psum", bufs=4, space="PSUM") as psum,
        ):
            # ---- constants ------------------------------------------------
            ident = const.tile([P, P], BF16, tag="ident")
            make_identity(nc, ident)
            boT = const.tile([P, KO], F32, tag="boT")
            nc.sync.dma_start(boT[:], bo_t)
            b1T = const.tile([P, FC], F32, tag="b1T")
            nc.sync.dma_start(b1T[:], b1_t)
            b2_bc = const.tile([P, DIM], BF16, tag="b2_bc")
            nc.gpsimd.dma_start(b2_bc[:], b2_ext[:][None, :].to_broadcast((P, DIM)))
            srow = const.tile([HEADS, LQ], F32, tag="srow")    # sumexp rows
    ln32 = const.tile([P, 1], F32, tag="ln32")         # exp bias: ln(1/32)
    nc.any.memset(ln32[:], LN_INV32)
            recip = const.tile([HEADS, LQ], BF16, tag="recip")
        
            def mm_psum():
                return psum.tile([P, 512], F32, tag="mm", name="mmps")

            # ---- load activations -----------------------------------------
            xT8 = pX8.tile([P, KO, LQ], FP8, tag="x8")         # x^T fp8
            nc.sync.dma_start(xT8[:], xT8_t)
            xT = pXT.tile([P, KO, LQ], BF16, tag="xtok")       # x^T bf16 (resid)
            nc.sync.dma_start(xT[:], xT_t)
            ctxT = pA.tile([P, KO, LK], FP8, tag="a4")         # ctx^T fp8
            nc.sync.dma_start(ctxT[:], ctxT_t)

            def load_weight(src_t):
                w_sb = wres.tile([P, KO, INNER], FP8, tag="w2", name="w_sb")
                for ko in range(KO):
                    nc.sync.dma_start(w_sb[:, ko, :], src_t[:, ko, :])
                return w_sb

            # ---- Q^T = Wq^T @ x^T  (feature-major, fp8 DoubleRow) ---------
            wq_sb = load_weight(wq_t)
            QT = pQT.tile([P, IC, LQ], BF16, tag="qt")
            for ic in range(IC):
                ps = mm_psum()
                for k2 in range(KO // 2):
                    nc.tensor.matmul(
                        ps[:], wq_sb[:, 2 * k2:2 * k2 + 2, ic * P:(ic + 1) * P],
                        xT8[:, 2 * k2:2 * k2 + 2, :],
                        start=(k2 == 0), stop=(k2 == KO // 2 - 1), perf_mode=DR)
                nc.vector.tensor_copy(out=QT[:, ic, :], in_=ps[:])

            wk_sb = load_weight(wk_t)
    for k4 in range(LK // 512):
        nc.sync.dma_start(ctxT[:, :, k4 * 512:(k4 + 1) * 512], ctxT_t[k4])
    wv_sb = load_weight(wv_t)
    KT = pKT.tile([P, IC, LK], BF16, tag="kt")
    V4 = pV.tile([P, KC, HEADS, DH + 1], FP8, tag="v4")
    nc.any.memset(V4[:, :, :, DH:DH + 1], 1.0)
    OTu = pOT.tile([P, IC, LQ], BF16, tag="ot")        # unnormalized O^T
    OT8 = pO8.tile([P, IC, LQ], FP8, tag="o8")         # normalized, fp8

    def kproj(ic, k4):
        ps = mm_psum()
        for k2 in range(KO // 2):
            nc.tensor.matmul(
                ps[:], wk_sb[:, 2 * k2:2 * k2 + 2, ic * P:(ic + 1) * P],
                ctxT[:, 2 * k2:2 * k2 + 2, k4 * 512:(k4 + 1) * 512],
                start=(k2 == 0), stop=(k2 == KO // 2 - 1), perf_mode=DR)
        nc.vector.tensor_copy(
            out=KT[:, ic, k4 * 512:(k4 + 1) * 512], in_=ps[:])

    def vproj(half, kp):
        ps = mm_psum()
        for k2 in range(KO // 2):
            nc.tensor.matmul(
                ps[:], ctxT[:, 2 * k2:2 * k2 + 2, kp * P:(kp + 1) * P],
                wv_sb[:, 2 * k2:2 * k2 + 2, half * 512:(half + 1) * 512],
                start=(k2 == 0), stop=(k2 == KO // 2 - 1), perf_mode=DR)
        nc.vector.tensor_copy(
            out=V4[:, kp, half * 8:(half + 1) * 8, 0:DH],
            in_=ps[:].rearrange("p (h d) -> p h d", d=DH))

    def qk_head(h):
        ic = h // 2
        pts = []
        for g in range(4):
            ptg = pPT.tile([P, 4, LQ], FP8, tag="pt", name="ptg")
            for jp in range(2):
                ps_s = mm_psum2()
                for jj in range(2):
                    kc = g * 4 + 2 * jp + jj
                    nc.tensor.matmul(
                        ps_s[:, jj, :], KT[:, ic, kc * P:(kc + 1) * P],
                        QTz[:, ic, h % 2, :], start=True, stop=True)
                nc.scalar.activation(
                    out=ptg[:, 2 * jp:2 * jp + 2, :], in_=ps_s[:], func=AF.Exp,
                    scale=SCALE, bias=ln32[:, 0:1])
            pts.append(ptg)
        return pts

    def pv_head(h, pts):
        po = DH * (h % 2)
        ic = h // 2
        ps_o = mm_psum()
        for kc2 in range(KC // 2):
            g, j = kc2 // 2, (kc2 % 2) * 2
            nc.tensor.matmul(
                ps_o[0:DH + 1, :], V4[:, 2 * kc2:2 * kc2 + 2, h, :],
                pts[g][:, j:j + 2, :],
                start=(kc2 == 0), stop=(kc2 == KC // 2 - 1), perf_mode=DR)
        nc.vector.tensor_copy(out=OTu[po:po + DH, ic, :], in_=ps_o[0:DH, :])
        stmp = rp.tile([1, LQ], F32, tag="st", name="stmp")
        nc.vector.tensor_copy(out=stmp[:], in_=ps_o[DH:DH + 1, :])
        nc.vector.reciprocal_approx_fast(out=stmp[:], in_=stmp[:])
        rb = rp.tile([1, LQ], BF16, tag="sr", name="rb")
        nc.vector.tensor_copy(out=rb[:], in_=stmp[:])
        return rb

    def norm_ic(ic, rb_even, rb_odd):
        # replicate the two per-head recip rows across partitions with
        # K=1 matmuls (PE idle in this window; avoids SWDGE DMA latency),
        # then normalize both heads of the ic chunk in one multiply
        ps_r = mm_psum()
        nc.tensor.matmul(ps_r[0:DH, :], ones512[:, 0:DH], rb_even[:],
                         start=True, stop=True)
        nc.tensor.matmul(ps_r[DH:P, :], ones512[:, 0:DH], rb_odd[:],
                         start=True, stop=True)
        nc.vector.tensor_mul(
            out=OT8[:, ic, :], in0=OTu[:, ic, :], in1=ps_r[:])

    # Software-pipelined attention: one head per slot, PV delayed two
    # slots, K/Q/V projections as fillers inside the stalls left by the
    # scalar engine's exp throughput.
    fillers = {
        0: [("k", 1, 0), ("k", 1, 1)] + [("v", 0, kp) for kp in range(8)],
        1: [("k", 1, 2), ("k", 1, 3), ("q", 1)] + [("v", 0, kp) for kp in range(8, 16)],
    }
    for i in range(2, 10):
        ic = i // 2 + 1
        fl = [("k", ic, 2 * (i % 2)), ("k", ic, 2 * (i % 2) + 1)]
        if i % 2 == 0:
            fl.append(("q", ic))
        fl += [("v", 1, 2 * (i - 2)), ("v", 1, 2 * (i - 2) + 1)]
        fillers[i] = fl
    for i in range(10, 14):
        ic = i // 2 + 1
        fl = [("k", ic, 2 * (i % 2)), ("k", ic, 2 * (i % 2) + 1)]
        if i % 2 == 0:
            fl.append(("q", ic))
        fillers[i] = fl

    qproj(0)
    for k4 in range(4):
        kproj(0, k4)
    pts_q = {}
    for s in range(18):
        if s < 16:
            pts_q[s] = qk_head(s)
        if s >= 2:
            h = s - 2
            rb = pv_head(h, pts_q.pop(h))
            if h % 2 == 0:
                rb_prev = rb
            else:
                norm_ic(h // 2, rb_prev, rb)
        for f in fillers.get(s, []):
            if f[0] == "k":
                kproj(f[1], f[2])
            elif f[0] == "v":
                vproj(f[1], f[2])
            else:
                qproj(f[1])

    # ---- X1^T = Wo^T @ O^T + bo + x^T  (DoubleRow, bf16 out) ------
            wo_sb = load_weight(wo_t)
            X1Tb = pOT.tile([P, KO, LQ], BF16, tag="ot")   # reuses OTu slot
            for dc in range(KO):
                ps = mm_psum()
                for i2 in range(IC // 2):
                    nc.tensor.matmul(
                        ps[:], wo_sb[:, 2 * i2:2 * i2 + 2, dc * P:(dc + 1) * P],
                        OT8[:, 2 * i2:2 * i2 + 2, :],
                        start=(i2 == 0), stop=(i2 == IC // 2 - 1), perf_mode=DR)
                nc.vector.scalar_tensor_tensor(
                    out=X1Tb[:, dc, :], in0=ps[:], scalar=boT[:, dc:dc + 1],
                    in1=xT[:, dc, :], op0=mybir.AluOpType.add,
                    op1=mybir.AluOpType.add)

            # ---- X1 token-major via PE transpose of X1^T (bias+residual
            # already fused there) --------------------------------------
            X1tok = pXT.tile([P, QO, DIM], BF16, tag="xtok")   # reuses xT slot
            for dc in range(KO):
                for qo in range(QO):
                    pt = psum.tile([P, P], BF16, tag="mm", name="tps")
                    nc.tensor.transpose(
                        pt[:], X1Tb[:, dc, qo * P:(qo + 1) * P], ident[:])
                    nc.vector.tensor_copy(
                        out=X1tok[:, qo, dc * P:(dc + 1) * P], in_=pt[:])

            # ---- H^T = gelu(W1^T @ X1^T + b1)  (feature-major, bf16) ------
            HT0 = pA.tile([P, FC // 2, LQ], BF16, tag="a4")    # reuses ctxT slot
    HT1 = pHT.tile([P, FC // 2, LQ], BF16, tag="ht")

    def HTsl(fc):
        return HT0[:, fc, :] if fc < 16 else HT1[:, fc - 16, :]
            for w in range(4):
                ps8 = [mm_psum() for _ in range(8)]
                for ko in range(KO):
                    wb = wstr.tile([P, 1024], BF16, tag="wstr", name="w1b")
                    nc.sync.dma_start(wb[:], w1_t[:, ko, w * 1024:(w + 1) * 1024])
                    for f8 in range(8):
                        nc.tensor.matmul(
                            ps8[f8][:], wb[:, f8 * P:(f8 + 1) * P], X1Tb[:, ko, :],
                            start=(ko == 0), stop=(ko == KO - 1))
                for f8 in range(8):
                    fc = w * 8 + f8
                    nc.scalar.activation(
                        out=HT[:, fc, :], in_=ps8[f8][:], func=AF.Gelu,
                        bias=b1T[:, fc:fc + 1])

            # ---- Y = H @ W2 + b2 + X1  (token-major, f32 out) -------------
            ps_y = [mm_psum() for _ in range(8)]
            for ks in range(FC):
                wb = wstr.tile([P, DIM], BF16, tag="wstr", name="w2b")
                nc.sync.dma_start(wb[:], w2_t[:, ks, :])
                for qo in range(QO):
                    for d2 in range(2):
                        nc.tensor.matmul(
                            ps_y[qo * 2 + d2][:], HT[:, ks, qo * P:(qo + 1) * P],
                            wb[:, d2 * 512:(d2 + 1) * 512],
                            start=(ks == 0), stop=(ks == FC - 1))
            for qo in range(QO):
                for d2 in range(2):
                    sl = slice(d2 * 512, (d2 + 1) * 512)
                    osb = io32.tile([P, DIM], F32, tag="io32", name="osb")
                    nc.vector.tensor_add(
                        out=osb[:, 0:512], in0=ps_y[qo * 2 + d2][:], in1=b2_bc[:, sl])
                    nc.vector.tensor_add(
                        out=osb[:, 0:512], in0=osb[:, 0:512], in1=X1tok[:, qo, sl])
                    nc.sync.dma_start(out_t[:, qo, sl], osb[:, 0:512])

    return nc



_NC_CACHE = {}


def _get_nc():
    if "nc" not in _NC_CACHE:
        nc = bacc.Bacc("TRN2", target_bir_lowering=False, debug=False, num_devices=8)
        build(nc)
        nc.compile()
        _NC_CACHE["nc"] = nc
    return _NC_CACHE["nc"]


def make_in_maps(inputs):
    bf = ml_dtypes.bfloat16
    f8 = ml_dtypes.float8_e4m3
    f = lambda a: np.asarray(a, dtype=np.float32)

    def pmaj(aT, dt):
        """[DIM, N] -> partition-major [P, KO, N]."""
        return np.ascontiguousarray(
            aT.astype(dt).reshape(KO, P, -1).transpose(1, 0, 2))

    x = f(inputs["x"]); ctx = f(inputs["context"])
    shared = {
        "Wq": pmaj(f(inputs["Wq"]), f8),
        "Wk": pmaj(f(inputs["Wk"]), f8),
        "Wv": pmaj(f(inputs["Wv"]), f8),
        "Wo": pmaj(f(inputs["Wo"]), f8),
        "W1": np.ascontiguousarray(f(inputs["W1"]).astype(bf)),
        "W2": np.ascontiguousarray(f(inputs["W2"]).astype(bf)),
        "bo": np.ascontiguousarray(f(inputs["bo"])),
        "b1": np.ascontiguousarray(f(inputs["b1"])),
        "b2": np.ascontiguousarray(f(inputs["b2"])),
    }
    # ctx^T -> [k4, P, KO, 512] so each column chunk is one fat-descriptor DMA
    ctxT_b = [np.ascontiguousarray(
        ctx[b].T.astype(f8).reshape(KO, P, LK // 512, 512).transpose(2, 1, 0, 3))
        for b in range(4)]
    in_maps = []
    for c in range(8):
        b, qs = c // 2, (c % 2) * LQ
        m = dict(shared)
        xsT = x[b, qs:qs + LQ, :].T
        m["xT8"] = pmaj(xsT, f8)
        m["xT"] = pmaj(xsT, bf)
        m["ctxT8"] = ctxT_b[b]
        in_maps.append(m)
    return in_maps


def run_full(inputs, trace=False):
    """Run on all 8 cores. Returns (full_output [4,1024,1024] f32, exec_time_ns)."""
    from concourse.bass_utils import run_bass_kernel_spmd

    nc = _get_nc()
    in_maps = make_in_maps(inputs)
    res = run_bass_kernel_spmd(nc, in_maps, core_ids=list(range(8)), trace=trace)
    out = np.empty((4, 1024, 1024), dtype=np.float32)
    for c in range(8):
        b, qs = c // 2, (c % 2) * LQ
        out[b, qs:qs + LQ, :] = np.asarray(res.results[c]["outT"]).transpose(2, 1, 0).reshape(LQ, DIM).astype(np.float32)
    return out, res.exec_time_ns


def kernel(**inputs):
    return run_full(inputs)[0]


# revision 30
# speedup vs baseline: 1.0905x; 1.0905x over previous
"""Trainium2 Bass kernel for nn_AdaptedTransformerBlock (dense transformer block,
cross-attention + FFN) running SPMD on 8 NeuronCores.

Sharding (data parallel, zero collectives): core c handles batch c//2 and
query rows (c%2)*512..+512. K/V for a batch's context are computed
redundantly on the pair of cores sharing that batch.

Precision: the attention path (Q/K/V/O projections, P@V) runs in fp8-e4m3
with DoubleRow double-pumped matmuls; QK^T runs in bf16 (it is output-bound,
fp8 would not speed it up); the FFN runs in bf16 (fp8 there costs too much
accuracy). exp() is emitted with a 1/32 output scale folded into the
activation bias so softmax weights fit fp8 range; the scale cancels in the
sum-exp normalization. All accumulation is f32 in PSUM.
"""

import numpy as np
import ml_dtypes


import concourse.bass as bass
import concourse.mybir as mybir
import concourse.tile as tile
from concourse import bacc

F32 = mybir.dt.float32
BF16 = mybir.dt.bfloat16
FP8 = mybir.dt.float8e4
AF = mybir.ActivationFunctionType
DR = mybir.MatmulPerfMode.DoubleRow

P = 128
DIM = 1024
INNER = 1024
HEADS = 16
DH = 64
FF = 4096
LQ = 512          # queries per core
LK = 2048
KO = DIM // P     # 8 contraction subtiles over dim
IC = INNER // P   # 8 inner chunks
KC = LK // P      # 16 key-position chunks
QO = LQ // P      # 4 query chunks
FC = FF // P      # 32 ffn chunks
SCALE = 0.125     # 1/sqrt(64)
LN_INV32 = float(np.log(1.0 / 32.0))  # exp output scale, cancels in softmax


def build(nc: bass.Bass):
    # ---- kernel I/O -------------------------------------------------------
    # fp8 inputs are pre-shuffled on the host to partition-major layouts
    # so each partition's data is contiguous in DRAM (8KB descriptors
    # instead of 1KB — the initial loads are descriptor-rate-bound)
    xT8_ext = nc.dram_tensor("xT8", [P, KO, LQ], FP8, kind="ExternalInput")
    xT_ext = nc.dram_tensor("xT", [P, KO, LQ], BF16, kind="ExternalInput")
    ctxT_ext = nc.dram_tensor("ctxT8", [LK // 512, P, KO, 512], FP8, kind="ExternalInput")
    wq_ext = nc.dram_tensor("Wq", [P, KO, INNER], FP8, kind="ExternalInput")
    wk_ext = nc.dram_tensor("Wk", [P, KO, INNER], FP8, kind="ExternalInput")
    wv_ext = nc.dram_tensor("Wv", [P, KO, INNER], FP8, kind="ExternalInput")
    wo_ext = nc.dram_tensor("Wo", [P, KO, INNER], FP8, kind="ExternalInput")
    bo_ext = nc.dram_tensor("bo", [DIM], F32, kind="ExternalInput")
    w1_ext = nc.dram_tensor("W1", [DIM, FF], BF16, kind="ExternalInput")
    b1_ext = nc.dram_tensor("b1", [FF], F32, kind="ExternalInput")
    w2_ext = nc.dram_tensor("W2", [FF, DIM], BF16, kind="ExternalInput")
    b2_ext = nc.dram_tensor("b2", [DIM], F32, kind="ExternalInput")
    out_ext = nc.dram_tensor("outT", [P, KO, LQ], BF16, kind="ExternalOutput")

    xT8_t = xT8_ext[:]                                 # [128,8,512]
    xT_t = xT_ext[:]                                   # [128,8,512]
    ctxT_t = ctxT_ext[:]                               # [4,128,8,512]
    wq_t = wq_ext[:]                                   # [128,8,1024]
    wk_t = wk_ext[:]
    wv_t = wv_ext[:]
    wo_t = wo_ext[:]
    w1_t = w1_ext[:].rearrange("(ko p) f -> p ko f", p=P)      # [128,8,4096]
    w2_t = w2_ext[:].rearrange("(ks p) d -> p ks d", p=P)      # [128,32,1024]
    bo_t = bo_ext[:].rearrange("(c p) -> p c", p=P)            # [128,8]
    b1_t = b1_ext[:].rearrange("(c p) -> p c", p=P)            # [128,32]
    out_t = out_ext[:]                                 # [128,8,512] partition-major

    with tile.TileContext(nc) as tc:
        with (
            tc.tile_pool(name="const", bufs=1) as const,
            tc.tile_pool(name="dram", bufs=1, space="DRAM") as dram,
            tc.tile_pool(name="pA", bufs=1) as pA,
            tc.tile_pool(name="pKT", bufs=1) as pKT,
            tc.tile_pool(name="pV", bufs=1) as pV,
            tc.tile_pool(name="pQT", bufs=1) as pQT,
            tc.tile_pool(name="pXT", bufs=1) as pXT,
            tc.tile_pool(name="pX8", bufs=1) as pX8,
            tc.tile_pool(name="pPT", bufs=12) as pPT,
            tc.tile_pool(name="pOT", bufs=1) as pOT,
            tc.tile_pool(name="pO8", bufs=1) as pO8,
            tc.tile_pool(name="pHT", bufs=1) as pHT,
            tc.tile_pool(name="pX1", bufs=1) as pX1,
            tc.tile_pool(name="wres", bufs=3) as wres,
            tc.tile_pool(name="wstr", bufs=11) as wstr,
            tc.tile_pool(name="io32", bufs=1) as io32,
            tc.tile_pool(name="rp", bufs=2) as rp,
            tc.tile_pool(name="psum", bufs=4, space="PSUM") as psum,
        ):
            # ---- constants ------------------------------------------------
            ident = const.tile([P, P], BF16, tag="ident")
            make_identity(nc, ident)
            boT = const.tile([P, KO], F32, tag="boT")
            nc.sync.dma_start(boT[:], bo_t)
            b1T = const.tile([P, FC], F32, tag="b1T")
            nc.sync.dma_start(b1T[:], b1_t)
            b2_bc = const.tile([P, DIM], BF16, tag="b2_bc")
            nc.gpsimd.dma_start(b2_bc[:], b2_ext[:][None, :].to_broadcast((P, DIM)))
            srow = const.tile([HEADS, LQ], F32, tag="srow")    # sumexp rows
    ln32 = const.tile([P, 1], F32, tag="ln32")         # exp bias: ln(1/32)
    nc.any.memset(ln32[:], LN_INV32)
            recip = const.tile([HEADS, LQ], BF16, tag="recip")
        
            def mm_psum():
                return psum.tile([P, 512], F32, tag="mm", name="mmps")

            # ---- load activations -----------------------------------------
            xT8 = pX8.tile([P, KO, LQ], FP8, tag="x8")         # x^T fp8
            nc.sync.dma_start(xT8[:], xT8_t)
            xT = pXT.tile([P, KO, LQ], BF16, tag="xtok")       # x^T bf16 (resid)
            nc.sync.dma_start(xT[:], xT_t)
            ctxT = pA.tile([P, KO, LK], FP8, tag="a4")         # ctx^T fp8
            nc.sync.dma_start(ctxT[:], ctxT_t)

            def load_weight(src_t):
                w_sb = wres.tile([P, KO, INNER], FP8, tag="w2", name="w_sb")
                for ko in range(KO):
                    nc.sync.dma_start(w_sb[:, ko, :], src_t[:, ko, :])
                return w_sb

            # ---- Q^T = Wq^T @ x^T  (feature-major, fp8 DoubleRow) ---------
            wq_sb = load_weight(wq_t)
            QT = pQT.tile([P, IC, LQ], BF16, tag="qt")
            for ic in range(IC):
                ps = mm_psum()
                for k2 in range(KO // 2):
                    nc.tensor.matmul(
                        ps[:], wq_sb[:, 2 * k2:2 * k2 + 2, ic * P:(ic + 1) * P],
                        xT8[:, 2 * k2:2 * k2 + 2, :],
                        start=(k2 == 0), stop=(k2 == KO // 2 - 1), perf_mode=DR)
                nc.vector.tensor_copy(out=QT[:, ic, :], in_=ps[:])

            wk_sb = load_weight(wk_t)
    for k4 in range(LK // 512):
        nc.sync.dma_start(ctxT[:, :, k4 * 512:(k4 + 1) * 512], ctxT_t[k4])
    wv_sb = load_weight(wv_t)
    KT = pKT.tile([P, IC, LK], BF16, tag="kt")
    V4 = pV.tile([P, KC, HEADS, DH + 1], FP8, tag="v4")
    nc.any.memset(V4[:, :, :, DH:DH + 1], 1.0)
    OTu = pOT.tile([P, IC, LQ], BF16, tag="ot")        # unnormalized O^T
    OT8 = pO8.tile([P, IC, LQ], FP8, tag="o8")         # normalized, fp8

    def kproj(ic, k4):
        ps = mm_psum()
        for k2 in range(KO // 2):
            nc.tensor.matmul(
                ps[:], wk_sb[:, 2 * k2:2 * k2 + 2, ic * P:(ic + 1) * P],
                ctxT[:, 2 * k2:2 * k2 + 2, k4 * 512:(k4 + 1) * 512],
                start=(k2 == 0), stop=(k2 == KO // 2 - 1), perf_mode=DR)
        nc.vector.tensor_copy(
            out=KT[:, ic, k4 * 512:(k4 + 1) * 512], in_=ps[:])

    def vproj(half, kp):
        ps = mm_psum()
        for k2 in range(KO // 2):
            nc.tensor.matmul(
                ps[:], ctxT[:, 2 * k2:2 * k2 + 2, kp * P:(kp + 1) * P],
                wv_sb[:, 2 * k2:2 * k2 + 2, half * 512:(half + 1) * 512],
                start=(k2 == 0), stop=(k2 == KO // 2 - 1), perf_mode=DR)
        nc.vector.tensor_copy(
            out=V4[:, kp, half * 8:(half + 1) * 8, 0:DH],
            in_=ps[:].rearrange("p (h d) -> p h d", d=DH))

    def emit_fill(f):
        if f[0] == "k":
            kproj(f[1], f[2])
        elif f[0] == "v":
            vproj(f[1], f[2])
        else:
            qproj(f[1])

    def qk_head(h, fills):
        # interleave filler projections between QK pairs: the scalar
        # engine drains exp slower than the PE issues pairs, so without
        # this the in-order PE queue stalls on PSUM rotation with filler
        # work parked behind it
        ic = h // 2
        pts = []
        fi = 0
        for g in range(4):
            ptg = pPT.tile([P, 4, LQ], FP8, tag="pt", name="ptg")
            for jp in range(2):
                ps_s = mm_psum2()
                for jj in range(2):
                    kc = g * 4 + 2 * jp + jj
                    nc.tensor.matmul(
                        ps_s[:, jj, :], KT[:, ic, kc * P:(kc + 1) * P],
                        QTz[:, ic, h % 2, :], start=True, stop=True)
                nc.scalar.activation(
                    out=ptg[:, 2 * jp:2 * jp + 2, :], in_=ps_s[:], func=AF.Exp,
                    scale=SCALE, bias=ln32[:, 0:1])
                if fi < len(fills):
                    emit_fill(fills[fi])
                    fi += 1
            pts.append(ptg)
        while fi < len(fills):
            emit_fill(fills[fi])
            fi += 1
        return pts

    def pv_head(h, pts):
        po = DH * (h % 2)
        ic = h // 2
        ps_o = mm_psum()
        for kc2 in range(KC // 2):
            g, j = kc2 // 2, (kc2 % 2) * 2
            nc.tensor.matmul(
                ps_o[0:DH + 1, :], V4[:, 2 * kc2:2 * kc2 + 2, h, :],
                pts[g][:, j:j + 2, :],
                start=(kc2 == 0), stop=(kc2 == KC // 2 - 1), perf_mode=DR)
        nc.vector.tensor_copy(out=OTu[po:po + DH, ic, :], in_=ps_o[0:DH, :])
        stmp = rp.tile([1, LQ], F32, tag="st", name="stmp")
        nc.vector.tensor_copy(out=stmp[:], in_=ps_o[DH:DH + 1, :])
        nc.vector.reciprocal_approx_fast(out=stmp[:], in_=stmp[:])
        rb = rp.tile([1, LQ], BF16, tag="sr", name="rb")
        nc.vector.tensor_copy(out=rb[:], in_=stmp[:])
        return rb

    def norm_ic(ic, rb_even, rb_odd):
        # replicate the two per-head recip rows across partitions with
        # K=1 matmuls (PE idle in this window; avoids SWDGE DMA latency),
        # then normalize both heads of the ic chunk in one multiply
        ps_r = mm_psum()
        nc.tensor.matmul(ps_r[0:DH, :], ones64[:], rb_even[:],
                         start=True, stop=True)
        nc.tensor.matmul(ps_r[DH:P, :], ones64[:], rb_odd[:],
                         start=True, stop=True)
        nc.vector.tensor_mul(
            out=OT8[:, ic, :], in0=OTu[:, ic, :], in1=ps_r[:])

    # Software-pipelined attention: one head per slot, PV delayed two
    # slots, K/Q/V projections as fillers inside the stalls left by the
    # scalar engine's exp throughput.
    fillers = {
        0: [("k", 1, 0), ("k", 1, 1)] + [("v", 0, kp) for kp in range(8)],
        1: [("k", 1, 2), ("k", 1, 3), ("q", 1)] + [("v", 0, kp) for kp in range(8, 16)],
    }
    for i in range(2, 10):
        ic = i // 2 + 1
        fl = [("k", ic, 2 * (i % 2)), ("k", ic, 2 * (i % 2) + 1)]
        if i % 2 == 0:
            fl.append(("q", ic))
        fl += [("v", 1, 2 * (i - 2)), ("v", 1, 2 * (i - 2) + 1)]
        fillers[i] = fl
    for i in range(10, 14):
        ic = i // 2 + 1
        fl = [("k", ic, 2 * (i % 2)), ("k", ic, 2 * (i % 2) + 1)]
        if i % 2 == 0:
            fl.append(("q", ic))
        fillers[i] = fl

    qproj(0)
    for k4 in range(4):
        kproj(0, k4)
    pts_q = {}
    for s in range(18):
        if s < 16:
            pts_q[s] = qk_head(s, fillers.get(s, []))
        else:
            for f in fillers.get(s, []):
                emit_fill(f)
        if s >= 2:
            h = s - 2
            rb = pv_head(h, pts_q.pop(h))
            if h % 2 == 0:
                rb_prev = rb
            else:
                norm_ic(h // 2, rb_prev, rb)

    # ---- X1^T = Wo^T @ O^T + bo + x^T  (DoubleRow, bf16 out) ------
            wo_sb = load_weight(wo_t)
            X1Tb = pOT.tile([P, KO, LQ], BF16, tag="ot")   # reuses OTu slot
            for dc in range(KO):
                ps = mm_psum()
                for i2 in range(IC // 2):
                    nc.tensor.matmul(
                        ps[:], wo_sb[:, 2 * i2:2 * i2 + 2, dc * P:(dc + 1) * P],
                        OT8[:, 2 * i2:2 * i2 + 2, :],
                        start=(i2 == 0), stop=(i2 == IC // 2 - 1), perf_mode=DR)
                nc.vector.scalar_tensor_tensor(
                    out=X1Tb[:, dc, :], in0=ps[:], scalar=boT[:, dc:dc + 1],
                    in1=xT[:, dc, :], op0=mybir.AluOpType.add,
                    op1=mybir.AluOpType.add)

            # ---- X1 token-major via PE transpose of X1^T (bias+residual
            # already fused there) --------------------------------------
            X1tok = pXT.tile([P, QO, DIM], BF16, tag="xtok")   # reuses xT slot
            for dc in range(KO):
                for qo in range(QO):
                    pt = psum.tile([P, P], BF16, tag="mm", name="tps")
                    nc.tensor.transpose(
                        pt[:], X1Tb[:, dc, qo * P:(qo + 1) * P], ident[:])
                    nc.vector.tensor_copy(
                        out=X1tok[:, qo, dc * P:(dc + 1) * P], in_=pt[:])

            # ---- H^T = gelu(W1^T @ X1^T + b1)  (feature-major, bf16) ------
            HT0 = pA.tile([P, FC // 2, LQ], BF16, tag="a4")    # reuses ctxT slot
    HT1 = pHT.tile([P, FC // 2, LQ], BF16, tag="ht")

    def HTsl(fc):
        return HT0[:, fc, :] if fc < 16 else HT1[:, fc - 16, :]
            for w in range(4):
                ps8 = [mm_psum() for _ in range(8)]
                for ko in range(KO):
                    wb = wstr.tile([P, 1024], BF16, tag="wstr", name="w1b")
                    nc.sync.dma_start(wb[:], w1_t[:, ko, w * 1024:(w + 1) * 1024])
                    for f8 in range(8):
                        nc.tensor.matmul(
                            ps8[f8][:], wb[:, f8 * P:(f8 + 1) * P], X1Tb[:, ko, :],
                            start=(ko == 0), stop=(ko == KO - 1))
                for f8 in range(8):
                    fc = w * 8 + f8
                    nc.scalar.activation(
                        out=HT[:, fc, :], in_=ps8[f8][:], func=AF.Gelu,
                        bias=b1T[:, fc:fc + 1])

            # ---- Y = H @ W2 + b2 + X1  (token-major, f32 out) -------------
            ps_y = [mm_psum() for _ in range(8)]
            for ks in range(FC):
                wb = wstr.tile([P, DIM], BF16, tag="wstr", name="w2b")
                nc.sync.dma_start(wb[:], w2_t[:, ks, :])
                for qo in range(QO):
                    for d2 in range(2):
                        nc.tensor.matmul(
                            ps_y[qo * 2 + d2][:], HT[:, ks, qo * P:(qo + 1) * P],
                            wb[:, d2 * 512:(d2 + 1) * 512],
                            start=(ks == 0), stop=(ks == FC - 1))
            for qo in range(QO):
                for d2 in range(2):
                    sl = slice(d2 * 512, (d2 + 1) * 512)
                    osb = io32.tile([P, DIM], F32, tag="io32", name="osb")
                    nc.vector.tensor_add(
                        out=osb[:, 0:512], in0=ps_y[qo * 2 + d2][:], in1=b2_bc[:, sl])
                    nc.vector.tensor_add(
                        out=osb[:, 0:512], in0=osb[:, 0:512], in1=X1tok[:, qo, sl])
                    nc.sync.dma_start(out_t[:, qo, sl], osb[:, 0:512])

    return nc



_NC_CACHE = {}


def _get_nc():
    if "nc" not in _NC_CACHE:
        nc = bacc.Bacc("TRN2", target_bir_lowering=False, debug=False, num_devices=8)
        build(nc)
        nc.compile()
        _NC_CACHE["nc"] = nc
    return _NC_CACHE["nc"]


def make_in_maps(inputs):
    bf = ml_dtypes.bfloat16
    f8 = ml_dtypes.float8_e4m3
    f = lambda a: np.asarray(a, dtype=np.float32)

    def pmaj(aT, dt):
        """[DIM, N] -> partition-major [P, KO, N]."""
        return np.ascontiguousarray(
            aT.astype(dt).reshape(KO, P, -1).transpose(1, 0, 2))

    x = f(inputs["x"]); ctx = f(inputs["context"])
    shared = {
        "Wq": pmaj(f(inputs["Wq"]), f8),
        "Wk": pmaj(f(inputs["Wk"]), f8),
        "Wv": pmaj(f(inputs["Wv"]), f8),
        "Wo": pmaj(f(inputs["Wo"]), f8),
        "W1": np.ascontiguousarray(f(inputs["W1"]).astype(bf)),
        "W2": np.ascontiguousarray(f(inputs["W2"]).astype(bf)),
        "bo": np.ascontiguousarray(f(inputs["bo"])),
        "b1": np.ascontiguousarray(f(inputs["b1"])),
        "b2": np.ascontiguousarray(f(inputs["b2"])),
    }
    # ctx^T -> [k4, P, KO, 512] so each column chunk is one fat-descriptor DMA
    ctxT_b = [np.ascontiguousarray(
        ctx[b].T.astype(f8).reshape(KO, P, LK // 512, 512).transpose(2, 1, 0, 3))
        for b in range(4)]
    in_maps = []
    for c in range(8):
        b, qs = c // 2, (c % 2) * LQ
        m = dict(shared)
        xsT = x[b, qs:qs + LQ, :].T
        m["xT8"] = pmaj(xsT, f8)
        m["xT"] = pmaj(xsT, bf)
        m["ctxT8"] = ctxT_b[b]
        in_maps.append(m)
    return in_maps


def run_full(inputs, trace=False):
    """Run on all 8 cores. Returns (full_output [4,1024,1024] f32, exec_time_ns)."""
    from concourse.bass_utils import run_bass_kernel_spmd

    nc = _get_nc()
    in_maps = make_in_maps(inputs)
    res = run_bass_kernel_spmd(nc, in_maps, core_ids=list(range(8)), trace=trace)
    out = np.empty((4, 1024, 1024), dtype=np.float32)
    for c in range(8):
        b, qs = c // 2, (c % 2) * LQ
        out[b, qs:qs + LQ, :] = np.asarray(res.results[c]["outT"]).transpose(2, 1, 0).reshape(LQ, DIM).astype(np.float32)
    return out, res.exec_time_ns


def kernel(**inputs):
    return run_full(inputs)[0]
